# revision 7
# baseline (speedup 1.0000x reference)
"""Trainium2 Bass kernel for nn_AttentionLayer (pooling attention).

Reference computation (S=2048, B=64, H=512):
    r      = (mask * sent).transpose(1,0,2)        # (B, S, H)
    WY     = r @ W
    WR     = mean_sent @ W_h
    M      = tanh(WY + WR[:, None, :])
    scores = M @ context                            # (B, S)
    alpha  = softmax(scores, axis=1)
    out    = sum_s alpha * r                        # (B, H)

Sharding: data-parallel over B across 8 cores (8 batches/core); W, W_h,
context replicated.

Design (three-way tanh split + 1.5-pass fp8 WY, ~45us steady state):
  - WY^T[k, s]: whole contraction in fp8 DoubleRow (pairs P0 = h 0:256,
    P1 = h 256:512) plus a W-residual DR pass (fp8(W - fp8(W)) on P0's
    rows) that rides the SAME rhs as P0 — 1.5 effective passes, no extra
    HBM traffic for the correction.
  - PSUM ring: three [128, 1024] slots (6 banks); each (b, kc) covers
    s=2048 across two slots, stitched into ONE activation read with a
    negative-stride 2D AP when the pair wraps the ring.
  - tanh split: ~23 (b, kc) tiles exact tanh on ACT (one [128, 2048]
    instruction each); ~9 tiles approximated on DVE as
    tanh(x) ~= sum_i a_i clamp(x, -t_i, t_i)  (4 clamps, rms err 4.8e-3)
    where each clamp runs at DVE 4x (594ns) and the weighted combine is
    FREE: each clamp plane enters the scores matmul as its own N=1
    accumulation pass against a_i-scaled ctx columns.
  - scores^T via N=1 matmuls (contraction over k on partitions), softmax
    without max-subtraction, final out^T = sum_c rn_block^T @ exp_col as
    in the 76.9us baseline; tails deferred into the next batch's WY
    stream at kc boundaries so tiny dependent matmuls never stall the
    in-order PE queue.
  - DMA lanes: SP carries rn (s-major bf16) for b0-5 + stores; gpsimd
    SWDGE carries rt8 (h-major fp8) + rn b6-7; ACT HWDGE carries the
    ~0.6MB of weights before its first tanh.  ~38us per lane.
  - batch 0 runs half-major with narrow tanhs off quarter loads and
    2-pass groups (no W-residual) so the pipe starts at ~4.5us; batch 7
    runs kc order (3,0,1,2) and splits scores into colA (first 3 kc,
    emitted under the last fills) + colB (last kc, drained after), so
    the post-stream drain is ~3us.
"""

import os
import numpy as np
import ml_dtypes

import concourse.bass as bass
import concourse.mybir as mybir
import concourse.tile as tile
from concourse import bacc, bass_utils
from concourse.ap import AP as BassAP

FP32 = mybir.dt.float32
BF16 = mybir.dt.bfloat16
FP8 = mybir.dt.float8e4

H = 512
S = 2048
B = 64
NCORES = 8
BPC = B // NCORES  # batches per core

KC = H // 128      # k chunks of 128 (output dim of W)
N_SC = S // 128    # 128-wide s chunks (score cols)

# 4-term clamp PWL fit of tanh on N(0, 0.816) input (rms err 4.8e-3)
PWL_T = [0.46433, 0.87163, 1.34208, 2.10719]
PWL_A = [0.295914, 0.300291, 0.232567, 0.127982]
NPL = len(PWL_T)

# (b, kc) tiles approximated on DVE; the rest run exact tanh on ACT.
# b0 all-exact (startup); b7's PWL on kc0 and kc order (3,0,1,2) so the
# last-drained tile is exact (short drain).
PWL_TILES = {(b, 3) for b in range(1, 7)} | {(3, 2), (4, 2), (7, 0)}

_cache = {}


def _build_nc(bpc=BPC, s=S):
    n_sc = s // 128
    st_n = s // 512
    nc = bacc.Bacc(None, target_bir_lowering=False)

    # ---- DRAM I/O ----
    # sent8[b, pair, t, p, s] = fp8(r[s, h]), h = (pair*2 + t)*128 + p
    sent8 = nc.dram_tensor("sent8", [bpc, 4 * 128, s], FP8, kind="ExternalInput")
    # s-major bf16 for the final weighted sum
    sentn = nc.dram_tensor("sentn", [bpc, s, H], BF16, kind="ExternalInput")
    # w8[pair, t, p, k] = fp8(W[h, k]); w8r = fp8(W[0:256] - w8[pair0])
    w8 = nc.dram_tensor("w8", [4 * 128, H], FP8, kind="ExternalInput")
    w8r = nc.dram_tensor("w8r", [2 * 128, H], FP8, kind="ExternalInput")
    # W_h bf16, split kc0 / kc1-3 for the first-bias critical path
    wh0 = nc.dram_tensor("wh0", [H, 128], BF16, kind="ExternalInput")
    whr = nc.dram_tensor("whr", [H, 3 * 128], BF16, kind="ExternalInput")
    meant = nc.dram_tensor("meant", [H, bpc], BF16, kind="ExternalInput")
    ctxv = nc.dram_tensor("ctxv", [H], BF16, kind="ExternalInput")
    out = nc.dram_tensor("out", [bpc, H], FP32, kind="ExternalOutput")

    with tile.TileContext(nc) as tc:
        with tc.tile_pool(name="singles", bufs=1) as singles, \
             tc.tile_pool(name="rt", bufs=2) as rt_pool, \
             tc.tile_pool(name="rn", bufs=3) as rn_pool, \
             tc.tile_pool(name="th", bufs=8) as th_pool, \
             tc.tile_pool(name="u", bufs=3) as u_pool, \
             tc.tile_pool(name="xb", bufs=2) as xb_pool, \
             tc.tile_pool(name="sm", bufs=2) as sm_pool, \
             tc.tile_pool(name="ring", bufs=1, space="PSUM") as ring_pool, \
             tc.tile_pool(name="mg", bufs=2, space="PSUM") as mg_pool:

            # ---- constants on the ACT HWDGE lane, ordered for the
            # startup critical path: w8 (first fills), wh0+meant (first
            # bias), then the rest ----
            w8_sb = singles.tile([128, 4 * H], FP8, tag="w8_sb")
            nc.scalar.dma_start(
                out=w8_sb.rearrange("p (a t k) -> p a t k", a=2, t=2),
                in_=w8.rearrange("(a t p) k -> p a t k", a=2, t=2),
            )
            wh0_sb = singles.tile([128, 4 * 128], BF16, tag="wh0_sb")
            nc.scalar.dma_start(
                out=wh0_sb.rearrange("p (hc k) -> p hc k", hc=4),
                in_=wh0.rearrange("(hc p) k -> p hc k", p=128),
            )
            meant_sb = singles.tile([128, 4 * bpc], BF16, tag="meant_sb")
            nc.scalar.dma_start(
                out=meant_sb.rearrange("p (hc b) -> p hc b", hc=4),
                in_=meant.rearrange("(hc p) b -> p hc b", p=128),
            )
            whr_sb = singles.tile([128, 4 * 3 * 128], BF16, tag="whr_sb")
            nc.scalar.dma_start(
                out=whr_sb.rearrange("p (hc k) -> p hc k", hc=4),
                in_=whr.rearrange("(hc p) k -> p hc k", p=128),
            )
            w8r_sb = singles.tile([128, 2 * H], FP8, tag="w8r_sb")
            nc.scalar.dma_start(
                out=w8r_sb.rearrange("p (t k) -> p t k", t=2),
                in_=w8r.rearrange("(t p) k -> p t k", t=2),
            )
            ctxT = singles.tile([128, KC], BF16, tag="ctxT")
            nc.scalar.dma_start(out=ctxT, in_=ctxv.rearrange("(c p) -> p c", p=128))

            w8_4d = w8_sb.rearrange("p (a t k) -> p a t k", a=2, t=2)
            w8r_3d = w8r_sb.rearrange("p (t k) -> p t k", t=2)

            # fp32 ones for partition-sum / broadcast matmuls; junk for
            # the PE p-state warmup
            ones_col = singles.tile([128, 1], FP32, tag="ones_col")
            nc.vector.memset(ones_col, 1.0)
            ones_row = singles.tile([1, 128], FP32, tag="ones_row")
            nc.vector.memset(ones_row, 1.0)
            junk = singles.tile([128, 512], BF16, tag="junk")
            nc.vector.memset(junk, 0.25)

            # ---- PSUM ring: 3 slots x [128, 1024] in one 6-bank tile ----
            ring = ring_pool.tile([128, 3072], FP32, tag="ring")

            def ring_pair_src(c):
                """[128, 2048] read AP over slots (c, (c+1)%3)."""
                if c == 0:
                    return ring[:, 0:2048]
                if c == 1:
                    return ring[:, 1024:3072]
                base = ring[:, 0:3072]
                return BassAP(base.tensor, base.offset + 2048,
                              [[3072, 128], [-2048, 2], [1, 1024]])

            def ring_dst(c, st):
                """512-col dst of stile st within the (c, c+1) pair."""
                slot = c if st < 2 else (c + 1) % 3
                off = slot * 1024 + (st % 2) * 512
                return ring[:, off: off + 512]

            # ---- PE warmup: junk matmuls bridge the p-state ramp ----
            for i in range(3):
                nc.tensor.matmul(ring[:, 0:512], lhsT=junk[:, 0:128],
                                 rhs=junk, start=(i == 0), stop=(i == 2))

            # ---- WR^T[k, b] on PE; biases copied to SBUF per kc ----
            wrT = singles.tile([128, KC * bpc], FP32, tag="wrT")
            wr_ps = mg_pool.tile([128, 48], FP32, tag="mg", name="wr_ps")
            wr_done = set()

            def emit_wr_chunk(kc):
                wr_done.add(kc)
                for hc in range(4):
                    if kc == 0:
                        lhsT = wh0_sb.rearrange(
                            "p (h k) -> p h k", h=4)[:, hc, :]
                    else:
                        lhsT = whr_sb.rearrange(
                            "p (h k) -> p h k", h=4)[:, hc,
                                                     (kc - 1) * 128: kc * 128]
                    nc.tensor.matmul(
                        wr_ps[:, kc * bpc: (kc + 1) * bpc],
                        lhsT=lhsT,
                        rhs=meant_sb.rearrange("p (h b) -> p h b", h=4)[:, hc, :],
                        start=(hc == 0),
                        stop=(hc == 3),
                    )
                nc.vector.tensor_copy(wrT[:, kc * bpc: (kc + 1) * bpc],
                                      wr_ps[:, kc * bpc: (kc + 1) * bpc])

            # a_i-scaled ctx columns for the PWL clamp planes
            ctxa = singles.tile([128, NPL * KC], BF16, tag="ctxa")
            ctxa_done = [False]

            def emit_ctxa():
                ctxa_done[0] = True
                for i in range(NPL):
                    nc.vector.tensor_scalar(
                        ctxa[:, i * KC: (i + 1) * KC], ctxT, float(PWL_A[i]),
                        None, mybir.AluOpType.mult)

            # ---- per-batch input loads ----
            rt_tiles = {}
            rn_tiles = {}

            def load_rt(b):
                rt8 = rt_pool.tile([128, 4 * s], FP8, tag="rt8", bufs=2,
                                   name=f"rt8_{b}")
                nc.gpsimd.dma_start(
                    out=rt8.rearrange("p (a t s) -> p a t s", a=2, t=2),
                    in_=sent8[b].rearrange("(a t p) s -> p a t s", a=2, t=2),
                )
                rt_tiles[b] = rt8

            def load_rn(b, engine):
                rnt = rn_pool.tile([128, n_sc * H], BF16, tag="rn", bufs=3,
                                   name=f"rn{b}")
                engine.dma_start(
                    out=rnt.rearrange("p (c h) -> p c h", c=n_sc),
                    in_=sentn[b].rearrange("(c p) h -> p c h", p=128),
                )
                rn_tiles[b] = rnt

            # batch 0 rt8 in stile quarters for an early start
            rt0_q = []
            src8 = sent8[0].rearrange("(a t p) s -> p a t s", a=2, t=2)
            for st in range(st_n):
                q8 = singles.tile([128, 4 * 512], FP8, tag=f"rt0q{st}",
                                  name=f"rt0q{st}")
                nc.gpsimd.dma_start(
                    out=q8.rearrange("p (a t s) -> p a t s", a=2, t=2),
                    in_=src8[:, :, :, st * 512: (st + 1) * 512],
                )
                rt0_q.append(q8)
            load_rn(0, nc.sync)

            # ---- WY group emission ----
            def emit_wy_group(b, kc, st, dst):
                """P0(start) [+ w8res] + P1(stop) DR matmuls into dst."""
                if b == 0:
                    rhs4 = rt0_q[st].rearrange("p (a t s) -> p a t s",
                                               a=2, t=2)
                    r_p0 = rhs4[:, 0, :, :]
                    r_p1 = rhs4[:, 1, :, :]
                else:
                    rhs4 = rt_tiles[b].rearrange("p (a t s) -> p a t s",
                                                 a=2, t=2)
                    r_p0 = rhs4[:, 0, :, st * 512: (st + 1) * 512]
                    r_p1 = rhs4[:, 1, :, st * 512: (st + 1) * 512]
                ksl = slice(kc * 128, (kc + 1) * 128)
                nc.tensor.matmul(
                    dst, lhsT=w8_4d[:, 0, :, ksl], rhs=r_p0,
                    start=True, stop=False,
                    perf_mode=mybir.MatmulPerfMode.DoubleRow)
                if b != 0:
                    nc.tensor.matmul(
                        dst, lhsT=w8r_3d[:, :, ksl], rhs=r_p0,
                        start=False, stop=False,
                        perf_mode=mybir.MatmulPerfMode.DoubleRow)
                nc.tensor.matmul(
                    dst, lhsT=w8_4d[:, 1, :, ksl], rhs=r_p1,
                    start=False, stop=True,
                    perf_mode=mybir.MatmulPerfMode.DoubleRow)

            # ---- deferred tails ----
            state = {}

            def bias_col(b, kc):
                return wrT[:, kc * bpc + b: kc * bpc + b + 1]

            def emit_scores(b, sc_list, kcs=range(KC), dst=None, dst_base=0):
                """scores^T[:, sc] = sum over kc (and clamp planes)."""
                if dst is None:
                    dst = state[("mg", b)]
                for sc in sc_list:
                    members = []
                    for kc in kcs:
                        if (b, kc) in PWL_TILES:
                            u = state[("u", b, kc)]
                            for i in range(NPL):
                                members.append((
                                    u[:, i * s + sc * 128: i * s + (sc + 1) * 128],
                                    ctxa[:, i * KC + kc: i * KC + kc + 1]))
                        else:
                            th = state[("th", b, kc)]
                            members.append((
                                th[:, sc * 128: (sc + 1) * 128],
                                ctxT[:, kc: kc + 1]))
                    nm = len(members)
                    for j, (lhsT, rhs) in enumerate(members):
                        nc.tensor.matmul(
                            dst[:, dst_base + sc: dst_base + sc + 1],
                            lhsT=lhsT, rhs=rhs,
                            start=(j == 0), stop=(j == nm - 1))

            def emit_exp_reduce(b, src=None):
                """exp + per-partition sums; src overrides the scores AP."""
                mg = state[("mg", b)]
                if src is None:
                    src = mg[:, 0:n_sc]
                expT = sm_pool.tile([128, n_sc], BF16, tag="expT", bufs=2,
                                    name=f"expT{b}")
                nc.scalar.activation(expT, src,
                                     mybir.ActivationFunctionType.Exp)
                accum = sm_pool.tile([128, 1], FP32, tag="accum", bufs=2,
                                     name=f"accum{b}")
                nc.vector.reduce_sum(
                    accum.rearrange("p (c o) -> p c o", o=1),
                    expT.rearrange("p (c s) -> p c s", c=1),
                    axis=mybir.AxisListType.X)
                state[("soft", b)] = (expT, accum)

            def emit_final_mms(b):
                """final 64 mms + sumexp mm + reciprocal."""
                expT, accum = state[("soft", b)]
                mg = state[("mg", b)]
                rnt = rn_tiles[b]
                for j in range(4):
                    for c in range(n_sc):
                        nc.tensor.matmul(
                            mg[:, 18 + j: 19 + j],
                            lhsT=rnt[:, c * H + j * 128: c * H + (j + 1) * 128],
                            rhs=expT[:, c: c + 1],
                            start=(c == 0), stop=(c == n_sc - 1))
                nc.tensor.matmul(mg[0:1, 16:17], lhsT=accum, rhs=ones_col,
                                 start=True, stop=True)
                rsum = sm_pool.tile([1, 1], FP32, tag="rsum", bufs=2,
                                    name=f"rsum{b}")
                nc.vector.reciprocal(rsum, mg[0:1, 16:17])
                state[("rsum", b)] = rsum

            def emit_store(b):
                """rsum broadcast + normalize + store; frees mg(b)."""
                mg = state.pop(("mg", b))
                rsum = state.pop(("rsum", b))
                state.pop(("soft", b))
                rn_tiles.pop(b)
                nc.tensor.matmul(mg[:, 17:18], lhsT=ones_row, rhs=rsum,
                                 start=True, stop=True)
                rsum_sb = sm_pool.tile([128, 1], FP32, tag="rsum_sb", bufs=2,
                                       name=f"rsum_sb{b}")
                nc.vector.tensor_copy(rsum_sb, mg[:, 17:18])
                out_sb = sm_pool.tile([128, 4], FP32, tag="out_sb", bufs=2,
                                      name=f"out_sb{b}")
                nc.vector.tensor_scalar_mul(out_sb, mg[:, 18:22], rsum_sb)
                nc.sync.dma_start(
                    out=out[b].rearrange("(j p) -> p j", p=128),
                    in_=out_sb)

            # ---- drains ----
            def drain_exact(b, kc, src, dst_off=0, width=None):
                if width is None:
                    width = s
                th = state.get(("th", b, kc))
                if th is None:
                    th = th_pool.tile([128, s], BF16, tag="th", bufs=8,
                                      name=f"th{b}_{kc}")
                    state[("th", b, kc)] = th
                nc.scalar.activation(
                    th[:, dst_off: dst_off + width],
                    src, mybir.ActivationFunctionType.Tanh,
                    bias=bias_col(b, kc), scale=1.0)

            def drain_pwl(b, kc, src):
                xb = xb_pool.tile([128, s], BF16, tag="xb", bufs=2,
                                  name=f"xb{b}_{kc}")
                nc.vector.tensor_scalar(xb, src, bias_col(b, kc), None,
                                        mybir.AluOpType.add)
                u = u_pool.tile([128, NPL * s], BF16, tag="u", bufs=3,
                                name=f"u{b}_{kc}")
                for i in range(NPL):
                    nc.vector.tensor_scalar(
                        u[:, i * s: (i + 1) * s], xb,
                        -float(PWL_T[i]), float(PWL_T[i]),
                        mybir.AluOpType.max, mybir.AluOpType.min)
                state[("u", b, kc)] = u

            # ================= main loop =================
            cursor = [0]

            def advance(k=1):
                c = cursor[0]
                cursor[0] = (c + k) % 3
                return c

            # ---- batch 0: half-major, narrow tanhs, 2-pass groups ----
            mg0 = mg_pool.tile([128, 48], FP32, tag="mg", name="mg0")
            state[("mg", 0)] = mg0
            load_rt(1)
            load_rn(1, nc.sync)
            for h in range(2):
                for kc in range(KC):
                    c = advance()
                    slot = ring[:, c * 1024: (c + 1) * 1024]
                    if kc not in wr_done:
                        emit_wr_chunk(kc)
                    if h == 0 and kc == 0:
                        # finest-grain start: tanh per stile
                        for sti in range(2):
                            emit_wy_group(0, kc, sti,
                                          slot[:, sti * 512: (sti + 1) * 512])
                            drain_exact(0, kc,
                                        slot[:, sti * 512: (sti + 1) * 512],
                                        dst_off=sti * 512, width=512)
                        continue
                    for sti in range(2):
                        emit_wy_group(0, kc, h * 2 + sti,
                                      slot[:, sti * 512: (sti + 1) * 512])
                    drain_exact(0, kc, slot, dst_off=h * 1024, width=1024)
                    if h == 1 and kc == 2:
                        emit_ctxa()

            # ---- batches 1..bpc-1 ----
            for b in range(1, bpc):
                if b + 1 < bpc:
                    load_rt(b + 1)
                    load_rn(b + 1, nc.sync if b + 1 <= 5 else nc.gpsimd)
                mg = mg_pool.tile([128, 48], FP32, tag="mg", name=f"mg{b}")
                state[("mg", b)] = mg
                kc_order = [3, 0, 1, 2] if b == bpc - 1 else [0, 1, 2, 3]
                for ki, kc in enumerate(kc_order):
                    c = advance(2)
                    for st in range(4):
                        emit_wy_group(b, kc, st, ring_dst(c, st))
                    if kc not in wr_done:
                        emit_wr_chunk(kc)
                    # deferred tail of batch b-1 (and b-2 finale) BEFORE
                    # the drain so the DVE softmax chain isn't queued
                    # behind PWL clamp work
                    q = b - 1
                    if ki == 0:
                        if b >= 2:
                            emit_store(b - 2)
                        emit_scores(q, range(0, 6))
                    elif ki == 1:
                        emit_scores(q, range(6, 12))
                    elif ki == 2:
                        emit_scores(q, range(12, 16))
                        emit_exp_reduce(q)
                    else:
                        emit_final_mms(q)
                        if b == bpc - 1:
                            # b7 colA: kc 3,0,1 into mg cols 0-15
                            emit_scores(b, range(16), kcs=[3, 0, 1])
                    src = ring_pair_src(c)
                    if (b, kc) in PWL_TILES:
                        drain_pwl(b, kc, src)
                    else:
                        drain_exact(b, kc, src)

            # ---- drain: b6 finale + b7 colB/softmax/final ----
            b = bpc - 1
            emit_store(b - 1)
            mg7 = state[("mg", b)]
            emit_scores(b, range(16), kcs=[2], dst=mg7, dst_base=22)
            scb = sm_pool.tile([128, n_sc], FP32, tag="scb", bufs=1,
                               name="scb")
            nc.vector.tensor_copy(scb, mg7[:, 22:38])
            sc7 = sm_pool.tile([128, n_sc], FP32, tag="sc7", bufs=1,
                               name="sc7")
            nc.vector.tensor_tensor(sc7, mg7[:, 0:16], scb,
                                    mybir.AluOpType.add)
            emit_exp_reduce(b, src=sc7)
            emit_final_mms(b)
            emit_store(b)

    nc.compile()
    return nc


def _get_nc(bpc, s):
    key = (bpc, s)
    if key not in _cache:
        _cache[key] = _build_nc(bpc, s)
    return _cache[key]


def _run(sent8, sentn, w8, w8r, wh_bf, meant, ctx_bf, ncores, bpc, s, **kw):
    nc = _get_nc(bpc, s)
    in_maps = []
    for c in range(ncores):
        in_maps.append({
            "sent8": sent8[c * bpc: (c + 1) * bpc],
            "sentn": sentn[c * bpc: (c + 1) * bpc],
            "w8": w8,
            "w8r": w8r,
            "wh0": np.ascontiguousarray(wh_bf[:, 0:128]),
            "whr": np.ascontiguousarray(wh_bf[:, 128:]),
            "meant": np.ascontiguousarray(
                meant[:, c * bpc: (c + 1) * bpc]),
            "ctxv": ctx_bf,
        })
    res = bass_utils.run_bass_kernel_spmd(nc, in_maps,
                                          core_ids=list(range(ncores)), **kw)
    outs = np.concatenate([res.results[c]["out"] for c in range(ncores)],
                          axis=0)
    return outs, res


def kernel(sent_batch, mean_sent_batch, batch_mask, W, W_h, context):
    sent_batch = np.asarray(sent_batch, dtype=np.float32)
    batch_mask = np.asarray(batch_mask, dtype=np.float32)
    mean_sent_batch = np.ascontiguousarray(
        np.asarray(mean_sent_batch, dtype=np.float32))
    W = np.asarray(W, dtype=np.float32)
    W_h = np.ascontiguousarray(np.asarray(W_h, dtype=np.float32))
    context = np.asarray(context, dtype=np.float32)

    if not np.all(batch_mask == 1.0):
        # general-correctness slow path; the mask is all-ones here
        sent_batch = sent_batch * batch_mask[:, :, None]

    bf16 = ml_dtypes.bfloat16
    fp8 = mybir.dt.np(FP8)

    sent_t = sent_batch.transpose(1, 2, 0)          # (B, H, S) view
    sent8 = np.ascontiguousarray(sent_t).astype(fp8)  # (B, H, S) fp8
    sent8 = sent8.reshape(B, 4 * 128, S)
    sentn = np.ascontiguousarray(
        sent_batch.astype(bf16).transpose(1, 0, 2))  # (B, S, H)

    w8 = W.astype(fp8)                               # (H, K)
    w8r = (W[0:256] - w8[0:256].astype(np.float32)).astype(fp8)
    wh_bf = W_h.astype(bf16)                         # (H, K)
    meant = np.ascontiguousarray(mean_sent_batch.T).astype(bf16)  # (H, B)
    ctx_bf = np.ascontiguousarray(context.astype(bf16))

    trace = bool(int(os.environ.get("KERNEL_TRACE", "0")))
    outs, res = _run(sent8, sentn, w8, w8r, wh_bf, meant, ctx_bf,
                     NCORES, BPC, S, trace=trace)
    kernel.last_results = res
    return outs.astype(np.float32)


kernel.last_results = None
